# revision 1
# baseline (speedup 1.0000x reference)
"""MGCN kernel for 8 trn2 NeuronCores.

Sharding (per spec hint): data-parallel over batch B=8 across the 8 cores.
The NxN adjacency/supports, the (N,C,C) weight pool and the embeddings are
replicated; each core computes the full model for its own batch element and
the host concatenates the per-core outputs back into the full (B,T,N,C)
tensor. All FLOPs run on the NeuronCores.
"""

import numpy as np
import jax
import jax.numpy as jnp

B, T, N, C, D = 8, 12, 1024, 64, 10


def _per_core(x_b, e1, e2, A_sym, weights_pool, bias_pool, alpha, beta, gamma):
    # x_b: (T, N, C) — this core's batch element.
    n = e1.shape[0]
    s = jnp.tanh(e1 @ e2.T - e2 @ e1.T)
    supports = jnp.eye(n, dtype=x_b.dtype) + jax.nn.relu(s)        # (N,N)

    A = jax.nn.softmax(A_sym, axis=-1)                             # (N,N)
    x_static = jax.nn.relu(jnp.einsum('nm,tmc->tnc', A, x_b))      # (T,N,C)

    # spatial attention; softmax over the TIME axis (axis=0 here)
    score = jnp.einsum('tnc,tmc->tnm', x_b, x_b)                   # (T,N,N)
    score = jax.nn.softmax(score, axis=0)
    x_sa = jax.nn.relu(jnp.einsum('tnm,tmc->tnc', score, x_b))     # (T,N,C)

    weights = jnp.einsum('nd,dio->nio', supports, weights_pool)    # (N,C,C)
    bias = supports @ bias_pool                                    # (N,C)
    x_g = jnp.einsum('nm,tmc->tnc', supports, x_b)                 # (T,N,C)
    x_gconv = jax.nn.relu(jnp.einsum('tni,nio->tno', x_g, weights) + bias)

    return alpha * x_gconv + beta * x_sa + gamma * x_static


_pmapped = jax.pmap(
    _per_core,
    in_axes=(0, None, None, None, None, None, None, None, None),
    devices=jax.devices()[:8],
)


def kernel(x, node_embeddings1, node_embeddings2, A_sym, weights_pool,
           bias_pool, alpha, beta, gamma):
    x = np.asarray(x, dtype=np.float32)
    out = _pmapped(
        x,  # (B=8, T, N, C) -> one batch element per core
        jnp.asarray(node_embeddings1, dtype=jnp.float32),
        jnp.asarray(node_embeddings2, dtype=jnp.float32),
        jnp.asarray(A_sym, dtype=jnp.float32),
        jnp.asarray(weights_pool, dtype=jnp.float32),
        jnp.asarray(bias_pool, dtype=jnp.float32),
        jnp.asarray(alpha, dtype=jnp.float32),
        jnp.asarray(beta, dtype=jnp.float32),
        jnp.asarray(gamma, dtype=jnp.float32),
    )
    return np.asarray(out, dtype=np.float32)


if __name__ == "__main__":
    rng = np.random.default_rng(0)
    ins = {
        "x": rng.standard_normal((B, T, N, C), dtype=np.float32),
        "node_embeddings1": rng.standard_normal((N, D), dtype=np.float32),
        "node_embeddings2": rng.standard_normal((N, D), dtype=np.float32),
        "A_sym": rng.random((N, N), dtype=np.float32),
        "weights_pool": rng.standard_normal((N, C, C), dtype=np.float32) * 0.02,
        "bias_pool": rng.standard_normal((N, C), dtype=np.float32) * 0.02,
        "alpha": np.array([0.9], dtype=np.float32),
        "beta": np.array([0.9], dtype=np.float32),
        "gamma": np.array([0.1], dtype=np.float32),
    }
    print(kernel(**ins).shape)

